# revision 45
# baseline (speedup 1.0000x reference)
"""Masked causal multi-head attention on 8 TRN2 NeuronCores.

Problem (hardcoded shapes): B=4, S=2048, D_MODEL=1024, HEADS=16,
KEY_SIZE=SIZE_PER_HEAD=64, OUT_DIM=1024, fp32 I/O.

Sharding: pure data/tensor parallel - core c handles batch b=c//2 and
head-group hg=c%2 (8 heads). Each core's output shard [2048, 512] is
independent, so there are no collectives; the host assembles shards.

Per-core pipeline (all matmul inputs bf16; PSUM accumulation fp32):
  - Key-side mask folded into the V path: host multiplies v rows by
    v_mask before the vw projection, and the fused denominator column
    (65th col of vw) holds v_mask instead of ones.  Masked keys then
    contribute exactly 0 to both the PV numerator and the denominator,
    so exp() needs NO per-k-tile bias -> logits stay O(1), no max
    subtraction needed, and one activation can cover both heads.
  - qc-outer schedule: loop q-chunks (512 wide) outermost, head-pairs
    inner.  Projection chains are emitted just-in-time, spread across
    the whole kernel so the PE never goes sparse and the HAM clock
    gate stays at k=8/8 (the original baseline lost 83us to a
    half-clock tail).  Emission order is execution order per engine
    queue, so every chain is placed after its input DMA lands but
    before its first consumer; dependency-free junk matmuls bridge
    DMA-bound stretches (lead-in, qc0->qc1 transition) and the
    exp-paced qc3 tail.
  - Per (pair, qc, kt): both heads' score matmuls (even head on PE
    rows 0-63, odd on 64-127) write one 2-bank PSUM tile; ONE
    activation exp(S/8) converts both to bf16 P (halves ACT
    instruction count - ACT is co-critical with the PE); causal
    masking of the diagonal block is a 0/1 multiply on P after exp;
    PV accumulates out^T [65, q] per head in PSUM.  PV is
    software-pipelined three k-tiles behind the scores - carried
    across pair AND q-chunk boundaries - so the exp latency stays off
    the PE's critical path and the exp stream never stalls behind a
    drain burst at a block transition.
  - Output staged bf16 (error budget allows it), both heads in one
    tile -> one DMA per (pair, qc).
  host: divide by denominators, multiply q_mask, transpose, assemble.
"""

import os
import sys

import numpy as np

for _p in ("/opt/trn_rl_repo",):
    if _p not in sys.path and os.path.isdir(_p):
        sys.path.insert(0, _p)

import ml_dtypes

import concourse.bass as bass
import concourse.mybir as mybir
import concourse.tile as tile
from concourse import bacc
from concourse.bass_utils import run_bass_kernel_spmd

B = 4
S = 2048
D = 1024
HEADS_PER_CORE = 8
DH = 64
HG_COLS = HEADS_PER_CORE * DH  # 512 output cols per core
NKT = S // 128  # 16 k-tiles
QC = 512  # q-chunk width (one PSUM bank)
NQC = S // QC

F32 = mybir.dt.float32
BF16 = mybir.dt.bfloat16
NP_BF16 = ml_dtypes.bfloat16

LAST_RESULT = None  # stashed BassKernelResults for test harness inspection
_NC_CACHE = None


def _ensure_ntff_hook():
    """The agent image's antenv lacks axon_hooks; synthesize it so
    run_bass_kernel_spmd(trace=True) can reach the NTFF profiler."""
    try:
        from antenv.axon_hooks import get_axon_ntff_profile_hook  # noqa: F401

        return
    except ImportError:
        pass
    import types

    try:
        import antenv
        from trn_agent_boot.trn_boot import _ntff_profile_via_ctypes
    except ImportError:
        return
    mod = types.ModuleType("antenv.axon_hooks")
    _hook = [None]
    try:
        _hook[0] = _ntff_profile_via_ctypes("/opt/axon/libaxon_pjrt.so")
    except OSError:
        pass
    mod.set_axon_ntff_profile_hook = lambda h: _hook.__setitem__(0, h)
    mod.get_axon_ntff_profile_hook = lambda: _hook[0]
    sys.modules["antenv.axon_hooks"] = mod
    antenv.axon_hooks = mod


def _build_nc() -> bass.Bass:
    nc = bacc.Bacc()

    xqT = nc.declare_dram_parameter("xqT", [D, S], BF16, isOutput=False)[:]
    xkT = nc.declare_dram_parameter("xkT", [D, S], BF16, isOutput=False)[:]
    xvT = nc.declare_dram_parameter("xvT", [D, S], BF16, isOutput=False)[:]
    wq = nc.declare_dram_parameter("wq", [D, HG_COLS], BF16, isOutput=False)[:]
    wk = nc.declare_dram_parameter("wk", [D, HG_COLS], BF16, isOutput=False)[:]
    wv = nc.declare_dram_parameter("wv", [D, HG_COLS], BF16, isOutput=False)[:]
    vmask = nc.declare_dram_parameter("vmask", [128, NKT], BF16, isOutput=False)[:]
    ctile = nc.declare_dram_parameter("ctile", [128, 128], BF16, isOutput=False)[:]
    outT = nc.declare_dram_parameter(
        "outT", [HEADS_PER_CORE * 65, S], BF16, isOutput=True
    )[:]

    with tile.TileContext(nc) as tc:
        with (
            tc.tile_pool(name="consts", bufs=1) as consts,
            tc.tile_pool(name="xqk", bufs=1) as xqk,
            tc.tile_pool(name="wpool", bufs=1) as wpool,
            tc.tile_pool(name="qk_sb", bufs=1) as qk_sb,
            tc.tile_pool(name="vw_pool", bufs=1) as vw_pool,
            tc.tile_pool(name="xvpool", bufs=1) as xvpool,
            tc.tile_pool(name="ppool", bufs=8) as ppool,
            tc.tile_pool(name="ostage", bufs=4) as ostage,
            tc.tile_pool(name="proj_ps", bufs=2, space="PSUM") as proj_ps,
            tc.tile_pool(name="att_s", bufs=2, space="PSUM") as spool,
            tc.tile_pool(name="att_oe", bufs=1, space="PSUM") as opool_e,
            tc.tile_pool(name="att_oo", bufs=1, space="PSUM") as opool_o,
        ):
            # PE warmup: dependency-free dummy matmuls run during the DMA
            # lead-in so the HAM clock gate opens (K=8/8) before the first
            # real matmul issues
            junk = consts.tile([128, 512], BF16, tag="junk")
            nc.vector.memset(junk, 0.0)
            for _ in range(36):
                wps = proj_ps.tile([128, 512], F32, tag="pp")
                nc.tensor.matmul(wps, junk[:, 0:128], junk, start=True, stop=True)

            # ---- DMA in need-order ----
            # constants first (tiny; vmask gates the vw denominator column,
            # ctile gates the first diagonal exp), then everything (pair0,
            # qc0) touches: wq+xq0 / wk+xk0 for the first score matmuls,
            # wv+xv0 for vw[0..3]; later chunks stream in behind.
            vmask_sb = consts.tile([128, NKT], BF16)
            nc.sync.dma_start(out=vmask_sb, in_=vmask)
            ctile_sb = consts.tile([128, 128], BF16)
            nc.sync.dma_start(out=ctile_sb, in_=ctile)

            xqT_r = xqT.rearrange("(t p) s -> p t s", p=128)
            xkT_r = xkT.rearrange("(t p) s -> p t s", p=128)
            xvT_r = xvT.rearrange("(t p) s -> p t s", p=128)
            wv_r = wv.rearrange("(t p) n -> t p n", p=128)

            # wq/wk split: pair0's weight columns land first so the very
            # first score matmuls gate on ~2.6MB of DMA instead of 4.1MB
            wq_r2 = wq.rearrange("(t p) n -> p t n", p=128)
            wk_r2 = wk.rearrange("(t p) n -> p t n", p=128)
            wq_sb = wpool.tile([128, 8, HG_COLS], BF16, tag="wq")
            nc.sync.dma_start(out=wq_sb[:, :, 0:128], in_=wq_r2[:, :, 0:128])
            xq_cs, xk_cs, xv_ts = [], [], []
            t_ = xqk.tile([128, 8, 512], BF16, tag="xq0")
            nc.sync.dma_start(out=t_, in_=xqT_r[:, :, 0:512])
            xq_cs.append(t_)
            wk_sb = wpool.tile([128, 8, HG_COLS], BF16, tag="wk")
            nc.sync.dma_start(out=wk_sb[:, :, 0:128], in_=wk_r2[:, :, 0:128])
            t_ = xqk.tile([128, 8, 512], BF16, tag="xk0")
            nc.sync.dma_start(out=t_, in_=xkT_r[:, :, 0:512])
            xk_cs.append(t_)
            nc.sync.dma_start(out=wq_sb[:, :, 128:HG_COLS], in_=wq_r2[:, :, 128:HG_COLS])
            nc.sync.dma_start(out=wk_sb[:, :, 128:HG_COLS], in_=wk_r2[:, :, 128:HG_COLS])

            wv_sb = wpool.tile([128, 8, HG_COLS], BF16, tag="wv")
            nc.sync.dma_start(out=wv_sb, in_=wv.rearrange("(t p) n -> p t n", p=128))
            xv_t = xvpool.tile([128, 8, 512], BF16, tag="xv0")
            nc.sync.dma_start(out=xv_t, in_=xvT_r[:, :, 0:512])
            xv_ts.append(xv_t)

            # later chunks stream in behind on the sync queue (issuing them
            # from the Activation queue was tried and blocks the exp stream:
            # DMA issues serialize on shared completion semaphores)
            for sc in range(1, NQC):
                for src, lst, tg, pool in (
                    (xqT_r, xq_cs, "xq", xqk),
                    (xkT_r, xk_cs, "xk", xqk),
                    (xvT_r, xv_ts, "xv", xvpool),
                ):
                    t_ = pool.tile([128, 8, 512], BF16, tag=f"{tg}{sc}")
                    nc.sync.dma_start(out=t_, in_=src[:, :, sc * 512 : (sc + 1) * 512])
                    lst.append(t_)

            # ---- SBUF-resident projected tensors ----
            qwT_sb = qk_sb.tile([128, 4, S], BF16)  # [dh%128, pair, s]
            kwT_sb = qk_sb.tile([128, 4, S], BF16)
            vw_sb = vw_pool.tile([128, NKT, HEADS_PER_CORE, 65], BF16)
            # denominator column = v_mask (masked keys contribute 0)
            for h in range(HEADS_PER_CORE):
                nc.vector.tensor_copy(
                    vw_sb[:, :, h, 64:65], vmask_sb.rearrange("p (k o) -> p k o", o=1)
                )

            # ---- projection emitters (each: one 8-matmul chain) ----
            def proj_qk_chain(w_sb, x_cs, dst, dt, sc):
                ps = proj_ps.tile([128, 512], F32, tag="pp")
                for t in range(8):
                    nc.tensor.matmul(
                        ps,
                        w_sb[:, t, dt * 128 : (dt + 1) * 128],
                        x_cs[sc][:, t, :],
                        start=(t == 0),
                        stop=(t == 7),
                    )
                nc.vector.tensor_copy(dst[:, dt, sc * 512 : (sc + 1) * 512], ps)

            def proj_vw_chain(kt):
                sc, st2 = kt // 4, kt % 4
                ps = proj_ps.tile([128, HG_COLS], F32, tag="pp")
                for t in range(8):
                    nc.tensor.matmul(
                        ps,
                        xv_ts[sc][:, t, st2 * 128 : (st2 + 1) * 128],
                        wv_sb[:, t, :],
                        start=(t == 0),
                        stop=(t == 7),
                    )
                nc.vector.tensor_copy(
                    vw_sb[:, kt, :, 0:64],
                    ps.rearrange("p (h d) -> p h d", h=HEADS_PER_CORE),
                )

            # ---- just-in-time projection schedule ----
            # Everything a q-chunk consumes is EMITTED at least one qc
            # earlier (the PE queue executes in emission order, so a
            # consumer emitted before its producer chain would block the
            # queue).  qc3 has nothing left to project; dependency-free
            # junk matmuls keep the PE dense there so the HAM clock gate
            # stays at k=8/8 through the exp-paced tail.
            def sched(qc):
                # job ORDER = emission order; chains must be emitted before
                # their consumers (in-order PE queue) but late enough that
                # their input DMA has landed (else they head-of-line block).
                jobs = []
                if qc == 0:
                    # handled explicitly in sched0() below
                    pass
                elif qc == 1:
                    # vw[4-7] first: consumed by qc1's own diagonal (~iter 6+)
                    for kt in range(4, 8):
                        jobs.append(("vw", kt))
                    for dt in range(4):
                        jobs.append(("qw", dt, 2))
                        jobs.append(("kw", dt, 2))
                    for kt in range(8, 12):
                        jobs.append(("vw", kt))
                elif qc == 2:
                    for dt in range(4):
                        jobs.append(("qw", dt, 3))
                        jobs.append(("kw", dt, 3))
                elif qc == 3:
                    # vw[12-15] ride in qc3's exp-shadow, emitted at
                    # 2-matmul granularity so each piece fits the per-
                    # iteration ACT slack (~300ns) instead of a 1.7us lump
                    # that idles the exp stream.  All four chains complete
                    # by iteration 15; first consumer is pair0's diagonal
                    # PV drain after iteration 15.  qc2 (PE-paced) sheds
                    # the 4 chains; junk fills the remaining iterations.
                    for kt in range(12, 16):
                        for tp in range(4):
                            jobs.append(("vwp", kt, tp))
                    # junk only where the exp stream is the pacer; the
                    # diagonal iterations (kt>=13: exp tiles shrink to
                    # 768/512/256 elems) are PE-paced, so junk there
                    # directly lengthens them
                    for pos in range(16, 64):
                        jobs.append(("junk",) if pos % 16 < 13 else ("nop",))
                return jobs

            vw_ps_open = {}

            def run_job(job):
                if job[0] == "qw":
                    proj_qk_chain(wq_sb, xq_cs, qwT_sb, job[1], job[2])
                elif job[0] == "kw":
                    proj_qk_chain(wk_sb, xk_cs, kwT_sb, job[1], job[2])
                elif job[0] == "vw":
                    proj_vw_chain(job[1])
                elif job[0] == "vwp":
                    # quarter of a vw chain: 2 accumulating matmuls
                    kt, tp = job[1], job[2]
                    sc, st2 = kt // 4, kt % 4
                    if tp == 0:
                        vw_ps_open[kt] = proj_ps.tile(
                            [128, HG_COLS], F32, tag="pp", name=f"vwp{kt}"
                        )
                    ps = vw_ps_open[kt]
                    for t in (2 * tp, 2 * tp + 1):
                        nc.tensor.matmul(
                            ps,
                            xv_ts[sc][:, t, st2 * 128 : (st2 + 1) * 128],
                            wv_sb[:, t, :],
                            start=(t == 0),
                            stop=(t == 7),
                        )
                    if tp == 3:
                        nc.vector.tensor_copy(
                            vw_sb[:, kt, :, 0:64],
                            vw_ps_open.pop(kt).rearrange(
                                "p (h d) -> p h d", h=HEADS_PER_CORE
                            ),
                        )
                elif job[0] == "junk":
                    # keeps PE issue density up, result unused
                    jps = proj_ps.tile([128, 512], F32, tag="pp")
                    nc.tensor.matmul(
                        jps, junk[:, 0:128], junk, start=True, stop=True
                    )
                # ("nop",): placeholder, emits nothing

            # minimal prologue: just pair0's first score matmuls.  vw[0-3]
            # (xv0-gated, lands ~24us) moves into qc0's schedule so the
            # earlier-landing score path isn't head-of-line blocked
            # behind it in the in-order PE queue.
            proj_qk_chain(wq_sb, xq_cs, qwT_sb, 0, 0)
            proj_qk_chain(wk_sb, xk_cs, kwT_sb, 0, 0)

            # Explicit qc0 schedule: pair p's sc0 chains before iteration
            # 4p; junk keeps the PE dense through DMA-gated stretches; the
            # sc1-dependent chains go at the very END of qc0 (their chunks
            # land ~20-25us in; emitting them earlier head-of-line blocks
            # the in-order PE queue while later, ready work starves).
            # qc0 carries only the remaining sc0 chains (pair p consumed
            # at iteration 4p).  The sc1-dependent chains go AFTER all of
            # qc0's attention emission (see post-qc0 block below): emitted
            # earlier they would sit ahead of qc0's later score matmuls in
            # the in-order PE queue and stall the exp stream while waiting
            # for the sc1 chunk DMAs (~21-24us).
            # vw[kt] is consumed by the lag-3 PV drain at iteration kt+3;
            # sc0 chains for pair p before iteration 4p
            sched0 = {
                0: [("qw", 1, 0), ("kw", 1, 0)],
                1: [("vw", 0)],
                2: [("vw", 1), ("qw", 2, 0)],
                3: [("vw", 2), ("kw", 2, 0)],
                4: [("vw", 3)],
                5: [("qw", 3, 0)], 6: [("kw", 3, 0)],
            }

            # PV software pipeline carried ACROSS pair AND q-chunk
            # boundaries: the previous block's last PVs drain during the
            # next block's first iterations, so the exp stream never
            # stalls behind a drain burst in the in-order PE queue.  A
            # pair's out copy + DMA are emitted when its last k-tile
            # drains; the next block's first PV (start=True) then
            # correctly waits on that copy (out pools have one bank each).
            pipe = []  # entries: (qc, q0, last_kt, pair, kt, q_off, p_t)
            outs = {}

            def emit_out(qc_, q0_, pr):
                out_e, out_o = outs.pop((qc_, pr))
                ost = ostage.tile([65, 2, QC], BF16, tag="ost")
                nc.vector.tensor_copy(ost[:, 0, :], out_e)
                nc.vector.tensor_copy(ost[:, 1, :], out_o)
                nc.sync.dma_start(
                    out=outT.rearrange("(h p) s -> p h s", p=65)[
                        :, 2 * pr : 2 * pr + 2, q0_ : q0_ + QC
                    ],
                    in_=ost,
                )

            def drain_one():
                qc_, q0_, lk, pr, kt_d, qo_d, p_d = pipe.pop(0)
                out_e, out_o = outs[(qc_, pr)]
                for h, out_ps, hh in ((0, out_e, 2 * pr), (1, out_o, 2 * pr + 1)):
                    nc.tensor.matmul(
                        out_ps[:, qo_d:QC],
                        vw_sb[:, kt_d, hh, :],
                        p_d[:, h, qo_d:QC],
                        start=(kt_d == 0),
                        stop=(kt_d == lk),
                    )
                if kt_d == lk:
                    emit_out(qc_, q0_, pr)

            # ---- main loop: q-chunks outer, head-pairs inner ----
            for qc in range(NQC):
                q0 = qc * QC
                last_kt = (q0 + QC) // 128 - 1
                n_iters = 4 * (last_kt + 1)
                if qc == 0:
                    job_at = sched0
                else:
                    jobs = sched(qc)
                    if not jobs:
                        jobs = [("junk",)] * n_iters
                    # spread jobs evenly across this qc's iterations
                    job_at = {}
                    for j, job in enumerate(jobs):
                        pos = (j * n_iters) // len(jobs)
                        job_at.setdefault(pos, []).append(job)
                it = 0

                for pair in range(4):
                    dt = pair
                    outs[(qc, pair)] = (
                        opool_e.tile(
                            [65, QC], F32, tag="oute", name=f"oe{qc}{pair}"
                        ),
                        opool_o.tile(
                            [65, QC], F32, tag="outo", name=f"oo{qc}{pair}"
                        ),
                    )

                    for kt in range(last_kt + 1):
                        q_off = max(0, 128 * kt - q0)
                        s_t = spool.tile([128, 2, QC], F32, tag="s")
                        for poff, h in ((0, 0), (64, 1)):
                            nc.tensor.matmul(
                                s_t[:, h, q_off:QC],
                                kwT_sb[
                                    poff : poff + 64, dt, kt * 128 : (kt + 1) * 128
                                ],
                                qwT_sb[poff : poff + 64, dt, q0 + q_off : q0 + QC],
                                start=True,
                                stop=True,
                            )
                        p_t = ppool.tile([128, 2, QC], BF16, tag="p")
                        nc.scalar.activation(
                            p_t[:, :, q_off:QC],
                            s_t[:, :, q_off:QC],
                            mybir.ActivationFunctionType.Exp,
                            scale=0.125,
                        )
                        if 128 * kt >= q0:
                            # causal mask as 0/1 multiply AFTER exp
                            for h in (0, 1):
                                nc.vector.tensor_mul(
                                    p_t[:, h, q_off : q_off + 128],
                                    p_t[:, h, q_off : q_off + 128],
                                    ctile_sb,
                                )
                        pipe.append((qc, q0, last_kt, pair, kt, q_off, p_t))
                        if len(pipe) > 3:
                            drain_one()
                        for job in job_at.get(it, ()):
                            run_job(job)
                        it += 1

                if qc == 0:
                    # post-qc0 bridge: junk carries the PE from qc0's
                    # drain (~18us) to the sc1-chunk DMA landing, then the
                    # sc1 chains feed qc1's first iterations.  qw first
                    # (consumed at qc1 iter 0), kw after (iter 4+).
                    for _ in range(16):
                        run_job(("junk",))
                    for dt in range(4):
                        run_job(("qw", dt, 1))
                    for dt in range(4):
                        run_job(("kw", dt, 1))
            while pipe:
                drain_one()
    nc.finalize()
    return nc


def _core_inputs(q, k, v, v_mask, Wq, Wk, Wv, b, hg):
    cols = slice(hg * HG_COLS, (hg + 1) * HG_COLS)
    # fold the key mask into the V path: masked keys' vw rows become 0
    vm = v[b] * v_mask[b][:, None]
    # causal 0/1 keep-mask for the diagonal block of S^T[k, q]: keep k <= q
    ct = (np.arange(128)[:, None] <= np.arange(128)[None, :]).astype(NP_BF16)
    return {
        "xqT": np.ascontiguousarray(q[b].T).astype(NP_BF16),
        "xkT": np.ascontiguousarray(k[b].T).astype(NP_BF16),
        "xvT": np.ascontiguousarray(vm.T).astype(NP_BF16),
        "wq": np.ascontiguousarray(Wq[:, cols]).astype(NP_BF16),
        "wk": np.ascontiguousarray(Wk[:, cols]).astype(NP_BF16),
        "wv": np.ascontiguousarray(Wv[:, cols]).astype(NP_BF16),
        "vmask": np.ascontiguousarray(
            v_mask[b].reshape(NKT, 128).T.astype(NP_BF16)
        ),
        "ctile": ct,
    }


def kernel(q, k, v, v_mask, q_mask, Wq, Wk, Wv):
    global LAST_RESULT, _NC_CACHE
    q = np.asarray(q, np.float32)
    k = np.asarray(k, np.float32)
    v = np.asarray(v, np.float32)
    v_mask = np.asarray(v_mask, np.float32)
    q_mask = np.asarray(q_mask, np.float32)
    Wq = np.asarray(Wq, np.float32)
    Wk = np.asarray(Wk, np.float32)
    Wv = np.asarray(Wv, np.float32)

    if _NC_CACHE is None:
        _NC_CACHE = _build_nc()
    nc = _NC_CACHE

    in_maps = [
        _core_inputs(q, k, v, v_mask, Wq, Wk, Wv, c // 2, c % 2) for c in range(8)
    ]
    _ensure_ntff_hook()
    res = run_bass_kernel_spmd(nc, in_maps, core_ids=list(range(8)))
    LAST_RESULT = res

    out = np.empty((B, S, D), np.float32)
    for c in range(8):
        b, hg = c // 2, c % 2
        o = np.asarray(res.results[c]["outT"], np.float32)  # [520, 2048]
        for h in range(HEADS_PER_CORE):
            pv = o[h * 65 : h * 65 + 64, :]  # [64, S]
            sm = o[h * 65 + 64, :]  # [S]
            sm = np.where(sm == 0.0, 1.0, sm)
            g = hg * HEADS_PER_CORE + h
            out[b, :, g * 64 : (g + 1) * 64] = (pv / sm).T
    out *= q_mask[:, :, None]

    # Degenerate rows: every causally-visible key masked. The reference's
    # additive -1e10 masks then make softmax uniform over all keys with
    # v_mask=1 (causality ignored). Patch on host; never triggers unless
    # v_mask[b, 0] == 0.
    for b in range(B):
        n_pref = int(np.argmax(v_mask[b] > 0)) if v_mask[b].max() > 0 else S
        if v_mask[b, 0] == 0 and n_pref > 0:
            vw_avg = ((v_mask[b] @ v[b]) / v_mask[b].sum()) @ Wv  # [OUT_DIM]
            out[b, :n_pref, :] = vw_avg[None, :] * q_mask[b, :n_pref, None]
    return out


# revision 46
# speedup vs baseline: 1.0249x; 1.0249x over previous
"""Masked causal multi-head attention on 8 TRN2 NeuronCores.

Problem (hardcoded shapes): B=4, S=2048, D_MODEL=1024, HEADS=16,
KEY_SIZE=SIZE_PER_HEAD=64, OUT_DIM=1024, fp32 I/O.

Sharding: pure data/tensor parallel - core c handles batch b=c//2 and
head-group hg=c%2 (8 heads). Each core's output shard [2048, 512] is
independent, so there are no collectives; the host assembles shards.

Per-core pipeline (all matmul inputs bf16; PSUM accumulation fp32):
  - Key-side mask folded into the V path: host multiplies v rows by
    v_mask before the vw projection, and the fused denominator column
    (65th col of vw) holds v_mask instead of ones.  Masked keys then
    contribute exactly 0 to both the PV numerator and the denominator,
    so exp() needs NO per-k-tile bias -> logits stay O(1), no max
    subtraction needed, and one activation can cover both heads.
  - qc-outer schedule: loop q-chunks (512 wide) outermost, head-pairs
    inner.  Projection chains are emitted just-in-time, spread across
    the whole kernel so the PE never goes sparse and the HAM clock
    gate stays at k=8/8 (the original baseline lost 83us to a
    half-clock tail).  Emission order is execution order per engine
    queue, so every chain is placed after its input DMA lands but
    before its first consumer; dependency-free junk matmuls bridge
    DMA-bound stretches (lead-in, qc0->qc1 transition) and the
    exp-paced qc3 tail.
  - Per (pair, qc, kt): both heads' score matmuls (even head on PE
    rows 0-63, odd on 64-127) write one 2-bank PSUM tile; ONE
    activation exp(S/8) converts both to bf16 P (halves ACT
    instruction count - ACT is co-critical with the PE); causal
    masking of the diagonal block is a 0/1 multiply on P after exp;
    PV accumulates out^T [65, q] per head in PSUM.  PV is
    software-pipelined three k-tiles behind the scores - carried
    across pair AND q-chunk boundaries - so the exp latency stays off
    the PE's critical path and the exp stream never stalls behind a
    drain burst at a block transition.
  - Output staged bf16 (error budget allows it), both heads in one
    tile -> one DMA per (pair, qc).
  host: divide by denominators, multiply q_mask, transpose, assemble.
"""

import os
import sys

import numpy as np

for _p in ("/opt/trn_rl_repo",):
    if _p not in sys.path and os.path.isdir(_p):
        sys.path.insert(0, _p)

import ml_dtypes

import concourse.bass as bass
import concourse.mybir as mybir
import concourse.tile as tile
from concourse import bacc
from concourse.bass_utils import run_bass_kernel_spmd

B = 4
S = 2048
D = 1024
HEADS_PER_CORE = 8
DH = 64
HG_COLS = HEADS_PER_CORE * DH  # 512 output cols per core
NKT = S // 128  # 16 k-tiles
QC = 512  # q-chunk width (one PSUM bank)
NQC = S // QC

F32 = mybir.dt.float32
BF16 = mybir.dt.bfloat16
NP_BF16 = ml_dtypes.bfloat16

LAST_RESULT = None  # stashed BassKernelResults for test harness inspection
_NC_CACHE = None


def _ensure_ntff_hook():
    """The agent image's antenv lacks axon_hooks; synthesize it so
    run_bass_kernel_spmd(trace=True) can reach the NTFF profiler."""
    try:
        from antenv.axon_hooks import get_axon_ntff_profile_hook  # noqa: F401

        return
    except ImportError:
        pass
    import types

    try:
        import antenv
        from trn_agent_boot.trn_boot import _ntff_profile_via_ctypes
    except ImportError:
        return
    mod = types.ModuleType("antenv.axon_hooks")
    _hook = [None]
    try:
        _hook[0] = _ntff_profile_via_ctypes("/opt/axon/libaxon_pjrt.so")
    except OSError:
        pass
    mod.set_axon_ntff_profile_hook = lambda h: _hook.__setitem__(0, h)
    mod.get_axon_ntff_profile_hook = lambda: _hook[0]
    sys.modules["antenv.axon_hooks"] = mod
    antenv.axon_hooks = mod


def _build_nc() -> bass.Bass:
    nc = bacc.Bacc()

    xqT = nc.declare_dram_parameter("xqT", [D, S], BF16, isOutput=False)[:]
    xkT = nc.declare_dram_parameter("xkT", [D, S], BF16, isOutput=False)[:]
    xvT = nc.declare_dram_parameter("xvT", [D, S], BF16, isOutput=False)[:]
    wq = nc.declare_dram_parameter("wq", [D, HG_COLS], BF16, isOutput=False)[:]
    wk = nc.declare_dram_parameter("wk", [D, HG_COLS], BF16, isOutput=False)[:]
    wv = nc.declare_dram_parameter("wv", [D, HG_COLS], BF16, isOutput=False)[:]
    vmask = nc.declare_dram_parameter("vmask", [128, NKT], BF16, isOutput=False)[:]
    ctile = nc.declare_dram_parameter("ctile", [128, 128], BF16, isOutput=False)[:]
    outT = nc.declare_dram_parameter(
        "outT", [HEADS_PER_CORE * 65, S], BF16, isOutput=True
    )[:]

    with tile.TileContext(nc) as tc:
        with (
            tc.tile_pool(name="consts", bufs=1) as consts,
            tc.tile_pool(name="xqk", bufs=1) as xqk,
            tc.tile_pool(name="wpool", bufs=1) as wpool,
            tc.tile_pool(name="qk_sb", bufs=1) as qk_sb,
            tc.tile_pool(name="vw_pool", bufs=1) as vw_pool,
            tc.tile_pool(name="xvpool", bufs=1) as xvpool,
            tc.tile_pool(name="ppool", bufs=8) as ppool,
            tc.tile_pool(name="ostage", bufs=4) as ostage,
            tc.tile_pool(name="proj_ps", bufs=2, space="PSUM") as proj_ps,
            tc.tile_pool(name="att_s", bufs=2, space="PSUM") as spool,
            tc.tile_pool(name="att_oe", bufs=1, space="PSUM") as opool_e,
            tc.tile_pool(name="att_oo", bufs=1, space="PSUM") as opool_o,
        ):
            # PE warmup: dependency-free dummy matmuls run during the DMA
            # lead-in so the HAM clock gate opens (K=8/8) before the first
            # real matmul issues
            junk = consts.tile([128, 512], BF16, tag="junk")
            nc.vector.memset(junk, 0.0)
            for _ in range(36):
                wps = proj_ps.tile([128, 512], F32, tag="pp")
                nc.tensor.matmul(wps, junk[:, 0:128], junk, start=True, stop=True)

            # ---- DMA in need-order ----
            # constants first (tiny; vmask gates the vw denominator column,
            # ctile gates the first diagonal exp), then everything (pair0,
            # qc0) touches: wq+xq0 / wk+xk0 for the first score matmuls,
            # wv+xv0 for vw[0..3]; later chunks stream in behind.
            vmask_sb = consts.tile([128, NKT], BF16)
            nc.sync.dma_start(out=vmask_sb, in_=vmask)
            ctile_sb = consts.tile([128, 128], BF16)
            nc.sync.dma_start(out=ctile_sb, in_=ctile)

            xqT_r = xqT.rearrange("(t p) s -> p t s", p=128)
            xkT_r = xkT.rearrange("(t p) s -> p t s", p=128)
            xvT_r = xvT.rearrange("(t p) s -> p t s", p=128)
            wv_r = wv.rearrange("(t p) n -> t p n", p=128)

            wq_sb = wpool.tile([128, 8, HG_COLS], BF16, tag="wq")
            nc.sync.dma_start(out=wq_sb, in_=wq.rearrange("(t p) n -> p t n", p=128))
            xq_cs, xk_cs, xv_ts = [], [], []
            t_ = xqk.tile([128, 8, 512], BF16, tag="xq0")
            nc.sync.dma_start(out=t_, in_=xqT_r[:, :, 0:512])
            xq_cs.append(t_)
            wk_sb = wpool.tile([128, 8, HG_COLS], BF16, tag="wk")
            nc.sync.dma_start(out=wk_sb, in_=wk.rearrange("(t p) n -> p t n", p=128))
            t_ = xqk.tile([128, 8, 512], BF16, tag="xk0")
            nc.sync.dma_start(out=t_, in_=xkT_r[:, :, 0:512])
            xk_cs.append(t_)

            wv_sb = wpool.tile([128, 8, HG_COLS], BF16, tag="wv")
            nc.sync.dma_start(out=wv_sb, in_=wv.rearrange("(t p) n -> p t n", p=128))
            xv_t = xvpool.tile([128, 8, 512], BF16, tag="xv0")
            nc.sync.dma_start(out=xv_t, in_=xvT_r[:, :, 0:512])
            xv_ts.append(xv_t)

            # later chunks stream in behind on the sync queue (issuing them
            # from the Activation queue was tried and blocks the exp stream:
            # DMA issues serialize on shared completion semaphores)
            for sc in range(1, NQC):
                for src, lst, tg, pool in (
                    (xqT_r, xq_cs, "xq", xqk),
                    (xkT_r, xk_cs, "xk", xqk),
                    (xvT_r, xv_ts, "xv", xvpool),
                ):
                    t_ = pool.tile([128, 8, 512], BF16, tag=f"{tg}{sc}")
                    nc.sync.dma_start(out=t_, in_=src[:, :, sc * 512 : (sc + 1) * 512])
                    lst.append(t_)

            # ---- SBUF-resident projected tensors ----
            qwT_sb = qk_sb.tile([128, 4, S], BF16)  # [dh%128, pair, s]
            kwT_sb = qk_sb.tile([128, 4, S], BF16)
            vw_sb = vw_pool.tile([128, NKT, HEADS_PER_CORE, 65], BF16)
            # denominator column = v_mask (masked keys contribute 0)
            for h in range(HEADS_PER_CORE):
                nc.vector.tensor_copy(
                    vw_sb[:, :, h, 64:65], vmask_sb.rearrange("p (k o) -> p k o", o=1)
                )

            # ---- projection emitters (each: one 8-matmul chain) ----
            def proj_qk_chain(w_sb, x_cs, dst, dt, sc):
                ps = proj_ps.tile([128, 512], F32, tag="pp")
                for t in range(8):
                    nc.tensor.matmul(
                        ps,
                        w_sb[:, t, dt * 128 : (dt + 1) * 128],
                        x_cs[sc][:, t, :],
                        start=(t == 0),
                        stop=(t == 7),
                    )
                nc.vector.tensor_copy(dst[:, dt, sc * 512 : (sc + 1) * 512], ps)

            def proj_vw_chain(kt):
                sc, st2 = kt // 4, kt % 4
                ps = proj_ps.tile([128, HG_COLS], F32, tag="pp")
                for t in range(8):
                    nc.tensor.matmul(
                        ps,
                        xv_ts[sc][:, t, st2 * 128 : (st2 + 1) * 128],
                        wv_sb[:, t, :],
                        start=(t == 0),
                        stop=(t == 7),
                    )
                nc.vector.tensor_copy(
                    vw_sb[:, kt, :, 0:64],
                    ps.rearrange("p (h d) -> p h d", h=HEADS_PER_CORE),
                )

            # ---- just-in-time projection schedule ----
            # Everything a q-chunk consumes is EMITTED at least one qc
            # earlier (the PE queue executes in emission order, so a
            # consumer emitted before its producer chain would block the
            # queue).  qc3 has nothing left to project; dependency-free
            # junk matmuls keep the PE dense there so the HAM clock gate
            # stays at k=8/8 through the exp-paced tail.
            def sched(qc):
                # job ORDER = emission order; chains must be emitted before
                # their consumers (in-order PE queue) but late enough that
                # their input DMA has landed (else they head-of-line block).
                jobs = []
                if qc == 0:
                    # handled explicitly in sched0() below
                    pass
                elif qc == 1:
                    # vw[4-7] first: consumed by qc1's own diagonal (~iter 6+)
                    for kt in range(4, 8):
                        jobs.append(("vw", kt))
                    for dt in range(4):
                        jobs.append(("qw", dt, 2))
                        jobs.append(("kw", dt, 2))
                    for kt in range(8, 12):
                        jobs.append(("vw", kt))
                elif qc == 2:
                    for dt in range(4):
                        jobs.append(("qw", dt, 3))
                        jobs.append(("kw", dt, 3))
                elif qc == 3:
                    # vw[12-15] ride in qc3's exp-shadow, emitted at
                    # 2-matmul granularity so each piece fits the per-
                    # iteration ACT slack (~300ns) instead of a 1.7us lump
                    # that idles the exp stream.  All four chains complete
                    # by iteration 15; first consumer is pair0's diagonal
                    # PV drain after iteration 15.  qc2 (PE-paced) sheds
                    # the 4 chains; junk fills the remaining iterations.
                    for kt in range(12, 16):
                        for tp in range(4):
                            jobs.append(("vwp", kt, tp))
                    # junk only where the exp stream is the pacer; the
                    # diagonal iterations (kt>=13: exp tiles shrink to
                    # 768/512/256 elems) are PE-paced, so junk there
                    # directly lengthens them
                    for pos in range(16, 64):
                        jobs.append(("junk",) if pos % 16 < 13 else ("nop",))
                return jobs

            vw_ps_open = {}

            def run_job(job):
                if job[0] == "qw":
                    proj_qk_chain(wq_sb, xq_cs, qwT_sb, job[1], job[2])
                elif job[0] == "kw":
                    proj_qk_chain(wk_sb, xk_cs, kwT_sb, job[1], job[2])
                elif job[0] == "vw":
                    proj_vw_chain(job[1])
                elif job[0] == "vwp":
                    # quarter of a vw chain: 2 accumulating matmuls
                    kt, tp = job[1], job[2]
                    sc, st2 = kt // 4, kt % 4
                    if tp == 0:
                        vw_ps_open[kt] = proj_ps.tile(
                            [128, HG_COLS], F32, tag="pp", name=f"vwp{kt}"
                        )
                    ps = vw_ps_open[kt]
                    for t in (2 * tp, 2 * tp + 1):
                        nc.tensor.matmul(
                            ps,
                            xv_ts[sc][:, t, st2 * 128 : (st2 + 1) * 128],
                            wv_sb[:, t, :],
                            start=(t == 0),
                            stop=(t == 7),
                        )
                    if tp == 3:
                        nc.vector.tensor_copy(
                            vw_sb[:, kt, :, 0:64],
                            vw_ps_open.pop(kt).rearrange(
                                "p (h d) -> p h d", h=HEADS_PER_CORE
                            ),
                        )
                elif job[0] == "junk":
                    # keeps PE issue density up, result unused
                    jps = proj_ps.tile([128, 512], F32, tag="pp")
                    nc.tensor.matmul(
                        jps, junk[:, 0:128], junk, start=True, stop=True
                    )
                # ("nop",): placeholder, emits nothing

            # minimal prologue: just pair0's first score matmuls.  vw[0-3]
            # (xv0-gated, lands ~24us) moves into qc0's schedule so the
            # earlier-landing score path isn't head-of-line blocked
            # behind it in the in-order PE queue.
            proj_qk_chain(wq_sb, xq_cs, qwT_sb, 0, 0)
            proj_qk_chain(wk_sb, xk_cs, kwT_sb, 0, 0)

            # Explicit qc0 schedule: pair p's sc0 chains before iteration
            # 4p; junk keeps the PE dense through DMA-gated stretches; the
            # sc1-dependent chains go at the very END of qc0 (their chunks
            # land ~20-25us in; emitting them earlier head-of-line blocks
            # the in-order PE queue while later, ready work starves).
            # qc0 carries only the remaining sc0 chains (pair p consumed
            # at iteration 4p).  The sc1-dependent chains go AFTER all of
            # qc0's attention emission (see post-qc0 block below): emitted
            # earlier they would sit ahead of qc0's later score matmuls in
            # the in-order PE queue and stall the exp stream while waiting
            # for the sc1 chunk DMAs (~21-24us).
            # vw[kt] is consumed by the lag-3 PV drain at iteration kt+3;
            # sc0 chains for pair p before iteration 4p
            sched0 = {
                0: [("qw", 1, 0), ("kw", 1, 0)],
                1: [("vw", 0)],
                2: [("vw", 1), ("qw", 2, 0)],
                3: [("vw", 2), ("kw", 2, 0)],
                4: [("vw", 3)],
                5: [("qw", 3, 0)], 6: [("kw", 3, 0)],
            }

            # PV software pipeline carried ACROSS pair AND q-chunk
            # boundaries: the previous block's last PVs drain during the
            # next block's first iterations, so the exp stream never
            # stalls behind a drain burst in the in-order PE queue.  A
            # pair's out copy + DMA are emitted when its last k-tile
            # drains; the next block's first PV (start=True) then
            # correctly waits on that copy (out pools have one bank each).
            pipe = []  # entries: (qc, q0, last_kt, pair, kt, q_off, p_t)
            outs = {}

            def emit_out(qc_, q0_, pr):
                out_e, out_o = outs.pop((qc_, pr))
                ost = ostage.tile([65, 2, QC], BF16, tag="ost")
                nc.vector.tensor_copy(ost[:, 0, :], out_e)
                nc.vector.tensor_copy(ost[:, 1, :], out_o)
                nc.sync.dma_start(
                    out=outT.rearrange("(h p) s -> p h s", p=65)[
                        :, 2 * pr : 2 * pr + 2, q0_ : q0_ + QC
                    ],
                    in_=ost,
                )

            def drain_one():
                qc_, q0_, lk, pr, kt_d, qo_d, p_d = pipe.pop(0)
                out_e, out_o = outs[(qc_, pr)]
                for h, out_ps, hh in ((0, out_e, 2 * pr), (1, out_o, 2 * pr + 1)):
                    nc.tensor.matmul(
                        out_ps[:, qo_d:QC],
                        vw_sb[:, kt_d, hh, :],
                        p_d[:, h, qo_d:QC],
                        start=(kt_d == 0),
                        stop=(kt_d == lk),
                    )
                if kt_d == lk:
                    emit_out(qc_, q0_, pr)

            # ---- main loop: q-chunks outer, head-pairs inner ----
            for qc in range(NQC):
                q0 = qc * QC
                last_kt = (q0 + QC) // 128 - 1
                n_iters = 4 * (last_kt + 1)
                if qc == 0:
                    job_at = sched0
                else:
                    jobs = sched(qc)
                    if not jobs:
                        jobs = [("junk",)] * n_iters
                    # spread jobs evenly across this qc's iterations
                    job_at = {}
                    for j, job in enumerate(jobs):
                        pos = (j * n_iters) // len(jobs)
                        job_at.setdefault(pos, []).append(job)
                it = 0

                for pair in range(4):
                    dt = pair
                    outs[(qc, pair)] = (
                        opool_e.tile(
                            [65, QC], F32, tag="oute", name=f"oe{qc}{pair}"
                        ),
                        opool_o.tile(
                            [65, QC], F32, tag="outo", name=f"oo{qc}{pair}"
                        ),
                    )

                    for kt in range(last_kt + 1):
                        q_off = max(0, 128 * kt - q0)
                        s_t = spool.tile([128, 2, QC], F32, tag="s")
                        for poff, h in ((0, 0), (64, 1)):
                            nc.tensor.matmul(
                                s_t[:, h, q_off:QC],
                                kwT_sb[
                                    poff : poff + 64, dt, kt * 128 : (kt + 1) * 128
                                ],
                                qwT_sb[poff : poff + 64, dt, q0 + q_off : q0 + QC],
                                start=True,
                                stop=True,
                            )
                        p_t = ppool.tile([128, 2, QC], BF16, tag="p")
                        nc.scalar.activation(
                            p_t[:, :, q_off:QC],
                            s_t[:, :, q_off:QC],
                            mybir.ActivationFunctionType.Exp,
                            scale=0.125,
                        )
                        if 128 * kt >= q0:
                            # causal mask as 0/1 multiply AFTER exp
                            for h in (0, 1):
                                nc.vector.tensor_mul(
                                    p_t[:, h, q_off : q_off + 128],
                                    p_t[:, h, q_off : q_off + 128],
                                    ctile_sb,
                                )
                        pipe.append((qc, q0, last_kt, pair, kt, q_off, p_t))
                        if len(pipe) > 3:
                            drain_one()
                        for job in job_at.get(it, ()):
                            run_job(job)
                        it += 1

                if qc == 0:
                    # post-qc0 bridge: junk carries the PE from qc0's
                    # drain (~18us) to the sc1-chunk DMA landing, then the
                    # sc1 chains feed qc1's first iterations.  qw first
                    # (consumed at qc1 iter 0), kw after (iter 4+).
                    for _ in range(16):
                        run_job(("junk",))
                    for dt in range(4):
                        run_job(("qw", dt, 1))
                    for dt in range(4):
                        run_job(("kw", dt, 1))
            while pipe:
                drain_one()
    nc.finalize()
    return nc


def _core_inputs(q, k, v, v_mask, Wq, Wk, Wv, b, hg):
    cols = slice(hg * HG_COLS, (hg + 1) * HG_COLS)
    # fold the key mask into the V path: masked keys' vw rows become 0
    vm = v[b] * v_mask[b][:, None]
    # causal 0/1 keep-mask for the diagonal block of S^T[k, q]: keep k <= q
    ct = (np.arange(128)[:, None] <= np.arange(128)[None, :]).astype(NP_BF16)
    return {
        "xqT": np.ascontiguousarray(q[b].T).astype(NP_BF16),
        "xkT": np.ascontiguousarray(k[b].T).astype(NP_BF16),
        "xvT": np.ascontiguousarray(vm.T).astype(NP_BF16),
        "wq": np.ascontiguousarray(Wq[:, cols]).astype(NP_BF16),
        "wk": np.ascontiguousarray(Wk[:, cols]).astype(NP_BF16),
        "wv": np.ascontiguousarray(Wv[:, cols]).astype(NP_BF16),
        "vmask": np.ascontiguousarray(
            v_mask[b].reshape(NKT, 128).T.astype(NP_BF16)
        ),
        "ctile": ct,
    }


def kernel(q, k, v, v_mask, q_mask, Wq, Wk, Wv):
    global LAST_RESULT, _NC_CACHE
    q = np.asarray(q, np.float32)
    k = np.asarray(k, np.float32)
    v = np.asarray(v, np.float32)
    v_mask = np.asarray(v_mask, np.float32)
    q_mask = np.asarray(q_mask, np.float32)
    Wq = np.asarray(Wq, np.float32)
    Wk = np.asarray(Wk, np.float32)
    Wv = np.asarray(Wv, np.float32)

    if _NC_CACHE is None:
        _NC_CACHE = _build_nc()
    nc = _NC_CACHE

    in_maps = [
        _core_inputs(q, k, v, v_mask, Wq, Wk, Wv, c // 2, c % 2) for c in range(8)
    ]
    _ensure_ntff_hook()
    res = run_bass_kernel_spmd(nc, in_maps, core_ids=list(range(8)))
    LAST_RESULT = res

    out = np.empty((B, S, D), np.float32)
    for c in range(8):
        b, hg = c // 2, c % 2
        o = np.asarray(res.results[c]["outT"], np.float32)  # [520, 2048]
        for h in range(HEADS_PER_CORE):
            pv = o[h * 65 : h * 65 + 64, :]  # [64, S]
            sm = o[h * 65 + 64, :]  # [S]
            sm = np.where(sm == 0.0, 1.0, sm)
            g = hg * HEADS_PER_CORE + h
            out[b, :, g * 64 : (g + 1) * 64] = (pv / sm).T
    out *= q_mask[:, :, None]

    # Degenerate rows: every causally-visible key masked. The reference's
    # additive -1e10 masks then make softmax uniform over all keys with
    # v_mask=1 (causality ignored). Patch on host; never triggers unless
    # v_mask[b, 0] == 0.
    for b in range(B):
        n_pref = int(np.argmax(v_mask[b] > 0)) if v_mask[b].max() > 0 else S
        if v_mask[b, 0] == 0 and n_pref > 0:
            vw_avg = ((v_mask[b] @ v[b]) / v_mask[b].sum()) @ Wv  # [OUT_DIM]
            out[b, :n_pref, :] = vw_avg[None, :] * q_mask[b, :n_pref, None]
    return out


# revision 47
# speedup vs baseline: 1.2002x; 1.1710x over previous
"""Masked causal multi-head attention on 8 TRN2 NeuronCores.

Problem (hardcoded shapes): B=4, S=2048, D_MODEL=1024, HEADS=16,
KEY_SIZE=SIZE_PER_HEAD=64, OUT_DIM=1024, fp32 I/O.

Sharding: pure data/tensor parallel - core c handles batch b=c//2 and
head-group hg=c%2 (8 heads). Each core's output shard [2048, 512] is
independent, so there are no collectives; the host assembles shards.

Per-core pipeline (all matmul inputs bf16; PSUM accumulation fp32):
  - Key-side mask folded into the V path: host multiplies v rows by
    v_mask before the vw projection, and the fused denominator column
    (65th col of vw) holds v_mask instead of ones.  Masked keys then
    contribute exactly 0 to both the PV numerator and the denominator,
    so exp() needs NO per-k-tile bias -> logits stay O(1), no max
    subtraction needed, and one activation can cover both heads.
  - qc-outer schedule: loop q-chunks (512 wide) outermost, head-pairs
    inner.  Projection chains are emitted just-in-time, spread across
    the whole kernel so the PE never goes sparse and the HAM clock
    gate stays at k=8/8 (the original baseline lost 83us to a
    half-clock tail).  Emission order is execution order per engine
    queue, so every chain is placed after its input DMA lands but
    before its first consumer; dependency-free junk matmuls bridge
    DMA-bound stretches (lead-in, qc0->qc1 transition) and the
    exp-paced qc3 tail.
  - Per (pair, qc, kt): both heads' score matmuls (even head on PE
    rows 0-63, odd on 64-127) write one 2-bank PSUM tile; ONE
    activation exp(S/8) converts both to bf16 P (halves ACT
    instruction count - ACT is co-critical with the PE); causal
    masking of the diagonal block is a 0/1 multiply on P after exp;
    PV accumulates out^T [65, q] per head in PSUM.  PV is
    software-pipelined three k-tiles behind the scores - carried
    across pair AND q-chunk boundaries - so the exp latency stays off
    the PE's critical path and the exp stream never stalls behind a
    drain burst at a block transition.
  - Output staged bf16 (error budget allows it), both heads in one
    tile -> one DMA per (pair, qc).
  host: divide by denominators, multiply q_mask, transpose, assemble.
"""

import os
import sys

import numpy as np

for _p in ("/opt/trn_rl_repo",):
    if _p not in sys.path and os.path.isdir(_p):
        sys.path.insert(0, _p)

import ml_dtypes

import concourse.bass as bass
import concourse.mybir as mybir
import concourse.tile as tile
from concourse import bacc
from concourse.bass_utils import run_bass_kernel_spmd

B = 4
S = 2048
D = 1024
HEADS_PER_CORE = 8
DH = 64
HG_COLS = HEADS_PER_CORE * DH  # 512 output cols per core
NKT = S // 128  # 16 k-tiles
QC = 512  # q-chunk width (one PSUM bank)
NQC = S // QC

F32 = mybir.dt.float32
BF16 = mybir.dt.bfloat16
NP_BF16 = ml_dtypes.bfloat16

LAST_RESULT = None  # stashed BassKernelResults for test harness inspection
_NC_CACHE = None


def _ensure_ntff_hook():
    """The agent image's antenv lacks axon_hooks; synthesize it so
    run_bass_kernel_spmd(trace=True) can reach the NTFF profiler."""
    try:
        from antenv.axon_hooks import get_axon_ntff_profile_hook  # noqa: F401

        return
    except ImportError:
        pass
    import types

    try:
        import antenv
        from trn_agent_boot.trn_boot import _ntff_profile_via_ctypes
    except ImportError:
        return
    mod = types.ModuleType("antenv.axon_hooks")
    _hook = [None]
    try:
        _hook[0] = _ntff_profile_via_ctypes("/opt/axon/libaxon_pjrt.so")
    except OSError:
        pass
    mod.set_axon_ntff_profile_hook = lambda h: _hook.__setitem__(0, h)
    mod.get_axon_ntff_profile_hook = lambda: _hook[0]
    sys.modules["antenv.axon_hooks"] = mod
    antenv.axon_hooks = mod


def _build_nc() -> bass.Bass:
    nc = bacc.Bacc()

    xqT = nc.declare_dram_parameter("xqT", [D, S], BF16, isOutput=False)[:]
    xkT = nc.declare_dram_parameter("xkT", [D, S], BF16, isOutput=False)[:]
    xvT = nc.declare_dram_parameter("xvT", [D, S], BF16, isOutput=False)[:]
    wq = nc.declare_dram_parameter("wq", [D, HG_COLS], BF16, isOutput=False)[:]
    wk = nc.declare_dram_parameter("wk", [D, HG_COLS], BF16, isOutput=False)[:]
    wv = nc.declare_dram_parameter("wv", [D, HG_COLS], BF16, isOutput=False)[:]
    vmask = nc.declare_dram_parameter("vmask", [128, NKT], BF16, isOutput=False)[:]
    ctile = nc.declare_dram_parameter("ctile", [128, 128], BF16, isOutput=False)[:]
    outT = nc.declare_dram_parameter(
        "outT", [HEADS_PER_CORE * 65, S], BF16, isOutput=True
    )[:]

    with tile.TileContext(nc) as tc:
        with (
            tc.tile_pool(name="consts", bufs=1) as consts,
            tc.tile_pool(name="xqk", bufs=1) as xqk,
            tc.tile_pool(name="wpool", bufs=1) as wpool,
            tc.tile_pool(name="qk_sb", bufs=1) as qk_sb,
            tc.tile_pool(name="vw_pool", bufs=1) as vw_pool,
            tc.tile_pool(name="xvpool", bufs=1) as xvpool,
            tc.tile_pool(name="ppool", bufs=10) as ppool,
            tc.tile_pool(name="ostage", bufs=4) as ostage,
            tc.tile_pool(name="proj_ps", bufs=2, space="PSUM") as proj_ps,
            tc.tile_pool(name="att_s", bufs=2, space="PSUM") as spool,
            tc.tile_pool(name="att_oe", bufs=1, space="PSUM") as opool_e,
            tc.tile_pool(name="att_oo", bufs=1, space="PSUM") as opool_o,
        ):
            # PE warmup: dependency-free dummy matmuls run during the DMA
            # lead-in so the HAM clock gate opens (K=8/8) before the first
            # real matmul issues
            junk = consts.tile([128, 512], BF16, tag="junk")
            nc.vector.memset(junk, 0.0)
            for _ in range(36):
                wps = proj_ps.tile([128, 512], F32, tag="pp")
                nc.tensor.matmul(wps, junk[:, 0:128], junk, start=True, stop=True)

            # ---- DMA in need-order ----
            # constants first (tiny; vmask gates the vw denominator column,
            # ctile gates the first diagonal exp), then everything (pair0,
            # qc0) touches: wq+xq0 / wk+xk0 for the first score matmuls,
            # wv+xv0 for vw[0..3]; later chunks stream in behind.
            vmask_sb = consts.tile([128, NKT], BF16)
            nc.sync.dma_start(out=vmask_sb, in_=vmask)
            ctile_sb = consts.tile([128, 128], BF16)
            nc.sync.dma_start(out=ctile_sb, in_=ctile)

            xqT_r = xqT.rearrange("(t p) s -> p t s", p=128)
            xkT_r = xkT.rearrange("(t p) s -> p t s", p=128)
            xvT_r = xvT.rearrange("(t p) s -> p t s", p=128)
            wv_r = wv.rearrange("(t p) n -> t p n", p=128)

            wq_sb = wpool.tile([128, 8, HG_COLS], BF16, tag="wq")
            nc.sync.dma_start(out=wq_sb, in_=wq.rearrange("(t p) n -> p t n", p=128))
            xq_cs, xk_cs, xv_ts = [], [], []
            t_ = xqk.tile([128, 8, 512], BF16, tag="xq0")
            nc.sync.dma_start(out=t_, in_=xqT_r[:, :, 0:512])
            xq_cs.append(t_)
            wk_sb = wpool.tile([128, 8, HG_COLS], BF16, tag="wk")
            nc.sync.dma_start(out=wk_sb, in_=wk.rearrange("(t p) n -> p t n", p=128))
            t_ = xqk.tile([128, 8, 512], BF16, tag="xk0")
            nc.sync.dma_start(out=t_, in_=xkT_r[:, :, 0:512])
            xk_cs.append(t_)

            wv_sb = wpool.tile([128, 8, HG_COLS], BF16, tag="wv")
            nc.sync.dma_start(out=wv_sb, in_=wv.rearrange("(t p) n -> p t n", p=128))
            xv_t = xvpool.tile([128, 8, 512], BF16, tag="xv0")
            nc.sync.dma_start(out=xv_t, in_=xvT_r[:, :, 0:512])
            xv_ts.append(xv_t)

            # later chunks stream in behind on the sync queue (issuing them
            # from the Activation queue was tried and blocks the exp stream:
            # DMA issues serialize on shared completion semaphores)
            for sc in range(1, NQC):
                for src, lst, tg, pool in (
                    (xqT_r, xq_cs, "xq", xqk),
                    (xkT_r, xk_cs, "xk", xqk),
                    (xvT_r, xv_ts, "xv", xvpool),
                ):
                    t_ = pool.tile([128, 8, 512], BF16, tag=f"{tg}{sc}")
                    nc.sync.dma_start(out=t_, in_=src[:, :, sc * 512 : (sc + 1) * 512])
                    lst.append(t_)

            # ---- SBUF-resident projected tensors ----
            qwT_sb = qk_sb.tile([128, 4, S], BF16)  # [dh%128, pair, s]
            kwT_sb = qk_sb.tile([128, 4, S], BF16)
            vw_sb = vw_pool.tile([128, NKT, HEADS_PER_CORE, 65], BF16)
            # denominator column = v_mask (masked keys contribute 0)
            for h in range(HEADS_PER_CORE):
                nc.vector.tensor_copy(
                    vw_sb[:, :, h, 64:65], vmask_sb.rearrange("p (k o) -> p k o", o=1)
                )

            # ---- projection emitters (each: one 8-matmul chain) ----
            def proj_qk_chain(w_sb, x_cs, dst, dt, sc):
                ps = proj_ps.tile([128, 512], F32, tag="pp")
                for t in range(8):
                    nc.tensor.matmul(
                        ps,
                        w_sb[:, t, dt * 128 : (dt + 1) * 128],
                        x_cs[sc][:, t, :],
                        start=(t == 0),
                        stop=(t == 7),
                    )
                nc.vector.tensor_copy(dst[:, dt, sc * 512 : (sc + 1) * 512], ps)

            def proj_vw_chain(kt):
                sc, st2 = kt // 4, kt % 4
                ps = proj_ps.tile([128, HG_COLS], F32, tag="pp")
                for t in range(8):
                    nc.tensor.matmul(
                        ps,
                        xv_ts[sc][:, t, st2 * 128 : (st2 + 1) * 128],
                        wv_sb[:, t, :],
                        start=(t == 0),
                        stop=(t == 7),
                    )
                nc.vector.tensor_copy(
                    vw_sb[:, kt, :, 0:64],
                    ps.rearrange("p (h d) -> p h d", h=HEADS_PER_CORE),
                )

            # ---- just-in-time projection schedule ----
            # Everything a q-chunk consumes is EMITTED at least one qc
            # earlier (the PE queue executes in emission order, so a
            # consumer emitted before its producer chain would block the
            # queue).  qc3 has nothing left to project; dependency-free
            # junk matmuls keep the PE dense there so the HAM clock gate
            # stays at k=8/8 through the exp-paced tail.
            def sched(qc):
                # job ORDER = emission order; chains must be emitted before
                # their consumers (in-order PE queue) but late enough that
                # their input DMA has landed (else they head-of-line block).
                jobs = []
                if qc == 0:
                    # handled explicitly in sched0() below
                    pass
                elif qc == 1:
                    # vw[4-7] first: consumed by qc1's own diagonal (~iter 6+)
                    for kt in range(4, 8):
                        jobs.append(("vw", kt))
                    for dt in range(4):
                        jobs.append(("qw", dt, 2))
                        jobs.append(("kw", dt, 2))
                    for kt in range(8, 12):
                        jobs.append(("vw", kt))
                elif qc == 2:
                    for dt in range(4):
                        jobs.append(("qw", dt, 3))
                        jobs.append(("kw", dt, 3))
                elif qc == 3:
                    # vw[12-15] ride in qc3's exp-shadow, emitted at
                    # 2-matmul granularity so each piece fits the per-
                    # iteration ACT slack (~300ns) instead of a 1.7us lump
                    # that idles the exp stream.  All four chains complete
                    # by iteration 15; first consumer is pair0's diagonal
                    # PV drain after iteration 15.  qc2 (PE-paced) sheds
                    # the 4 chains; junk fills the remaining iterations.
                    for kt in range(12, 16):
                        for tp in range(4):
                            jobs.append(("vwp", kt, tp))
                    # junk only where the exp stream is the pacer; the
                    # diagonal iterations (kt>=13: exp tiles shrink to
                    # 768/512/256 elems) are PE-paced, so junk there
                    # directly lengthens them
                    for pos in range(16, 64):
                        jobs.append(("junk",) if pos % 16 < 13 else ("nop",))
                return jobs

            vw_ps_open = {}

            def run_job(job):
                if job[0] == "qw":
                    proj_qk_chain(wq_sb, xq_cs, qwT_sb, job[1], job[2])
                elif job[0] == "kw":
                    proj_qk_chain(wk_sb, xk_cs, kwT_sb, job[1], job[2])
                elif job[0] == "vw":
                    proj_vw_chain(job[1])
                elif job[0] == "vwp":
                    # quarter of a vw chain: 2 accumulating matmuls
                    kt, tp = job[1], job[2]
                    sc, st2 = kt // 4, kt % 4
                    if tp == 0:
                        vw_ps_open[kt] = proj_ps.tile(
                            [128, HG_COLS], F32, tag="pp", name=f"vwp{kt}"
                        )
                    ps = vw_ps_open[kt]
                    for t in (2 * tp, 2 * tp + 1):
                        nc.tensor.matmul(
                            ps,
                            xv_ts[sc][:, t, st2 * 128 : (st2 + 1) * 128],
                            wv_sb[:, t, :],
                            start=(t == 0),
                            stop=(t == 7),
                        )
                    if tp == 3:
                        nc.vector.tensor_copy(
                            vw_sb[:, kt, :, 0:64],
                            vw_ps_open.pop(kt).rearrange(
                                "p (h d) -> p h d", h=HEADS_PER_CORE
                            ),
                        )
                elif job[0] == "junk":
                    # keeps PE issue density up, result unused
                    jps = proj_ps.tile([128, 512], F32, tag="pp")
                    nc.tensor.matmul(
                        jps, junk[:, 0:128], junk, start=True, stop=True
                    )
                # ("nop",): placeholder, emits nothing

            # minimal prologue: just pair0's first score matmuls.  vw[0-3]
            # (xv0-gated, lands ~24us) moves into qc0's schedule so the
            # earlier-landing score path isn't head-of-line blocked
            # behind it in the in-order PE queue.
            proj_qk_chain(wq_sb, xq_cs, qwT_sb, 0, 0)
            proj_qk_chain(wk_sb, xk_cs, kwT_sb, 0, 0)

            # Explicit qc0 schedule: pair p's sc0 chains before iteration
            # 4p; junk keeps the PE dense through DMA-gated stretches; the
            # sc1-dependent chains go at the very END of qc0 (their chunks
            # land ~20-25us in; emitting them earlier head-of-line blocks
            # the in-order PE queue while later, ready work starves).
            # qc0 carries only the remaining sc0 chains (pair p consumed
            # at iteration 4p).  The sc1-dependent chains go AFTER all of
            # qc0's attention emission (see post-qc0 block below): emitted
            # earlier they would sit ahead of qc0's later score matmuls in
            # the in-order PE queue and stall the exp stream while waiting
            # for the sc1 chunk DMAs (~21-24us).
            # qc0 runs a DEEPER PV lag (7) so the xv0-gated vw chains can
            # sit at positions 5-8 - executing right as xv0 lands (~23us)
            # - while iterations 2-7's scores/exps flow ahead of them
            # unblocked.  sc0 chains for pair p before iteration 4p.
            sched0 = {
                0: [("qw", 1, 0), ("kw", 1, 0)],
                1: [("qw", 2, 0)], 2: [("kw", 2, 0)],
                3: [("qw", 3, 0)], 4: [("kw", 3, 0)],
                5: [("vw", 0)], 6: [("vw", 1)],
                7: [("vw", 2)], 8: [("vw", 3)],
            }

            # PV software pipeline carried ACROSS pair AND q-chunk
            # boundaries: the previous block's last PVs drain during the
            # next block's first iterations, so the exp stream never
            # stalls behind a drain burst in the in-order PE queue.  A
            # pair's out copy + DMA are emitted when its last k-tile
            # drains; the next block's first PV (start=True) then
            # correctly waits on that copy (out pools have one bank each).
            pipe = []  # entries: (qc, q0, last_kt, pair, kt, q_off, p_t)
            outs = {}

            def emit_out(qc_, q0_, pr):
                out_e, out_o = outs.pop((qc_, pr))
                ost = ostage.tile([65, 2, QC], BF16, tag="ost")
                nc.vector.tensor_copy(ost[:, 0, :], out_e)
                nc.vector.tensor_copy(ost[:, 1, :], out_o)
                nc.sync.dma_start(
                    out=outT.rearrange("(h p) s -> p h s", p=65)[
                        :, 2 * pr : 2 * pr + 2, q0_ : q0_ + QC
                    ],
                    in_=ost,
                )

            def drain_one():
                qc_, q0_, lk, pr, kt_d, qo_d, p_d = pipe.pop(0)
                out_e, out_o = outs[(qc_, pr)]
                for h, out_ps, hh in ((0, out_e, 2 * pr), (1, out_o, 2 * pr + 1)):
                    nc.tensor.matmul(
                        out_ps[:, qo_d:QC],
                        vw_sb[:, kt_d, hh, :],
                        p_d[:, h, qo_d:QC],
                        start=(kt_d == 0),
                        stop=(kt_d == lk),
                    )
                if kt_d == lk:
                    emit_out(qc_, q0_, pr)

            # ---- main loop: q-chunks outer, head-pairs inner ----
            for qc in range(NQC):
                q0 = qc * QC
                last_kt = (q0 + QC) // 128 - 1
                n_iters = 4 * (last_kt + 1)
                if qc == 0:
                    job_at = sched0
                else:
                    jobs = sched(qc)
                    if not jobs:
                        jobs = [("junk",)] * n_iters
                    # spread jobs evenly across this qc's iterations
                    job_at = {}
                    for j, job in enumerate(jobs):
                        pos = (j * n_iters) // len(jobs)
                        job_at.setdefault(pos, []).append(job)
                it = 0

                for pair in range(4):
                    dt = pair
                    outs[(qc, pair)] = (
                        opool_e.tile(
                            [65, QC], F32, tag="oute", name=f"oe{qc}{pair}"
                        ),
                        opool_o.tile(
                            [65, QC], F32, tag="outo", name=f"oo{qc}{pair}"
                        ),
                    )

                    for kt in range(last_kt + 1):
                        q_off = max(0, 128 * kt - q0)
                        s_t = spool.tile([128, 2, QC], F32, tag="s")
                        for poff, h in ((0, 0), (64, 1)):
                            nc.tensor.matmul(
                                s_t[:, h, q_off:QC],
                                kwT_sb[
                                    poff : poff + 64, dt, kt * 128 : (kt + 1) * 128
                                ],
                                qwT_sb[poff : poff + 64, dt, q0 + q_off : q0 + QC],
                                start=True,
                                stop=True,
                            )
                        p_t = ppool.tile([128, 2, QC], BF16, tag="p")
                        nc.scalar.activation(
                            p_t[:, :, q_off:QC],
                            s_t[:, :, q_off:QC],
                            mybir.ActivationFunctionType.Exp,
                            scale=0.125,
                        )
                        if 128 * kt >= q0:
                            # causal mask as 0/1 multiply AFTER exp
                            for h in (0, 1):
                                nc.vector.tensor_mul(
                                    p_t[:, h, q_off : q_off + 128],
                                    p_t[:, h, q_off : q_off + 128],
                                    ctile_sb,
                                )
                        pipe.append((qc, q0, last_kt, pair, kt, q_off, p_t))
                        while len(pipe) > (7 if qc == 0 else 3):
                            drain_one()
                        for job in job_at.get(it, ()):
                            run_job(job)
                        it += 1

                if qc == 0:
                    # post-qc0 bridge: junk carries the PE from qc0's
                    # drain (~18us) to the sc1-chunk DMA landing, then the
                    # sc1 chains feed qc1's first iterations.  qw first
                    # (consumed at qc1 iter 0), kw after (iter 4+).
                    for _ in range(16):
                        run_job(("junk",))
                    for dt in range(4):
                        run_job(("qw", dt, 1))
                    for dt in range(4):
                        run_job(("kw", dt, 1))
            while pipe:
                drain_one()
    nc.finalize()
    return nc


def _core_inputs(q, k, v, v_mask, Wq, Wk, Wv, b, hg):
    cols = slice(hg * HG_COLS, (hg + 1) * HG_COLS)
    # fold the key mask into the V path: masked keys' vw rows become 0
    vm = v[b] * v_mask[b][:, None]
    # causal 0/1 keep-mask for the diagonal block of S^T[k, q]: keep k <= q
    ct = (np.arange(128)[:, None] <= np.arange(128)[None, :]).astype(NP_BF16)
    return {
        "xqT": np.ascontiguousarray(q[b].T).astype(NP_BF16),
        "xkT": np.ascontiguousarray(k[b].T).astype(NP_BF16),
        "xvT": np.ascontiguousarray(vm.T).astype(NP_BF16),
        "wq": np.ascontiguousarray(Wq[:, cols]).astype(NP_BF16),
        "wk": np.ascontiguousarray(Wk[:, cols]).astype(NP_BF16),
        "wv": np.ascontiguousarray(Wv[:, cols]).astype(NP_BF16),
        "vmask": np.ascontiguousarray(
            v_mask[b].reshape(NKT, 128).T.astype(NP_BF16)
        ),
        "ctile": ct,
    }


def kernel(q, k, v, v_mask, q_mask, Wq, Wk, Wv):
    global LAST_RESULT, _NC_CACHE
    q = np.asarray(q, np.float32)
    k = np.asarray(k, np.float32)
    v = np.asarray(v, np.float32)
    v_mask = np.asarray(v_mask, np.float32)
    q_mask = np.asarray(q_mask, np.float32)
    Wq = np.asarray(Wq, np.float32)
    Wk = np.asarray(Wk, np.float32)
    Wv = np.asarray(Wv, np.float32)

    if _NC_CACHE is None:
        _NC_CACHE = _build_nc()
    nc = _NC_CACHE

    in_maps = [
        _core_inputs(q, k, v, v_mask, Wq, Wk, Wv, c // 2, c % 2) for c in range(8)
    ]
    _ensure_ntff_hook()
    res = run_bass_kernel_spmd(nc, in_maps, core_ids=list(range(8)))
    LAST_RESULT = res

    out = np.empty((B, S, D), np.float32)
    for c in range(8):
        b, hg = c // 2, c % 2
        o = np.asarray(res.results[c]["outT"], np.float32)  # [520, 2048]
        for h in range(HEADS_PER_CORE):
            pv = o[h * 65 : h * 65 + 64, :]  # [64, S]
            sm = o[h * 65 + 64, :]  # [S]
            sm = np.where(sm == 0.0, 1.0, sm)
            g = hg * HEADS_PER_CORE + h
            out[b, :, g * 64 : (g + 1) * 64] = (pv / sm).T
    out *= q_mask[:, :, None]

    # Degenerate rows: every causally-visible key masked. The reference's
    # additive -1e10 masks then make softmax uniform over all keys with
    # v_mask=1 (causality ignored). Patch on host; never triggers unless
    # v_mask[b, 0] == 0.
    for b in range(B):
        n_pref = int(np.argmax(v_mask[b] > 0)) if v_mask[b].max() > 0 else S
        if v_mask[b, 0] == 0 and n_pref > 0:
            vw_avg = ((v_mask[b] @ v[b]) / v_mask[b].sum()) @ Wv  # [OUT_DIM]
            out[b, :n_pref, :] = vw_avg[None, :] * q_mask[b, :n_pref, None]
    return out
